# revision 1
# baseline (speedup 1.0000x reference)
"""DPFP multi-head linear attention kernel for 8 Trainium2 NeuronCores.

Sharding: data-parallel over batch (B=2 -> two groups of 4 cores), tensor-
parallel over heads (16 heads -> 4 heads per core), row-sharded o-projection
with a chunked ReduceScatter over each 4-core group, residual+LayerNorm on
the scattered rows.

Math (per batch b, per head n):
  q = h @ Wq, k,v = split(h @ Wkv)
  pq = dpfp(q), pk = dpfp(k)          # dpfp: [relu(x), relu(-x)] then
                                      # concat_i x2 * roll(x2, i), i=1..3
  kvzT = [v | 1]^T-contract-S pk      # [D+1, F]; row D = z = sum_j pk
  numT/den = kvz-contract-F pqT       # [D+1, S] per head
  attn_vecT = numT / (den + EPS/SCALE)  # SCALE cancels between num and den
  out = LN(h + attn_vec @ Wo)

All matmuls run as float32r (fp32 data, tf32-like mantissa, full PE rate).
Set BASS_KERNEL_FP32=1 to force exact fp32 matmuls (4x slower PE).
"""
import contextlib
import os
import sys

sys.path.insert(0, "/opt/trn_rl_repo")

import numpy as np

import concourse.bacc as bacc
import concourse.mybir as mybir
import concourse.tile as tile
from concourse.bass_utils import run_bass_kernel_spmd

AF = mybir.ActivationFunctionType
ALU = mybir.AluOpType
F32 = mybir.dt.float32
F32R = mybir.dt.float32 if os.environ.get("BASS_KERNEL_FP32") else mybir.dt.float32r
# dtype for the dpfp feature map (x2, pk, pq, vext, kvz, pqT): bf16 halves DVE
# mul time and PE transpose time; the num/den ratio cancels the rounding.
FMAP = mybir.dt.bfloat16 if os.environ.get("BASS_KERNEL_FMAP_BF16") else F32R
QMAP = mybir.dt.bfloat16 if os.environ.get("BASS_KERNEL_QBF16") else F32R

S = 2048
B = 2
D = 1024
NH = 16          # total heads
NHC = 4          # heads per core
DH = 64
FD = 384         # dpfp feature dim per head = 2*DH*3
EPS_EFF = 8e-5   # EPS / SCALE = 1e-5 * sqrt(64)
DP = DH + 4      # padded [v|1] width for even PE-transpose dims
N_CORES = 8
GRP = 4          # cores per reduce-scatter group
SCH = S // 128   # 16 s-chunks of 128
SG = 4           # s-groups of 512
KC = D // 128    # 8 contraction chunks


def _emit_proj(nc, tc, ev, io, const, sb_hT, sb_hld, sb_qkv, ps):
    """h -> hT (transposed), then q/k/v projections in natural layout."""
    (h_in, _hres_in, _out_p, _rs_in, _rs_out, _r_dram) = io
    (wq_sb, wkv_sb, _wo_sb, _gb_sb, identr, _onesf, _epst, _epse, _onesrow, _identb, _identq) = const

    hT = sb_hT.tile([128, KC, S], F32R, tag="hT")
    for i in range(SCH):
        h_tile = sb_hld.tile([128, D], F32R, tag="hld", bufs=3)
        nc.gpsimd.dma_start(out=h_tile[:], in_=h_in[i * 128:(i + 1) * 128, :])
        for cpair in range(KC // 4):
            pt = ps.tile([128, 4, 128], F32R, tag="ps")
            for j in range(4):
                c = cpair * 4 + j
                nc.tensor.transpose(
                    pt[:, j, :], h_tile[:, c * 128:(c + 1) * 128], identr[:])
            ev()(
                hT[:, cpair * 4:cpair * 4 + 4, i * 128:(i + 1) * 128], pt[:])

    q_nat = sb_qkv.tile([128, SCH, NHC * DH], F32, tag="q_nat")
    kv_sb = sb_qkv.tile([128, SCH, 2 * NHC * DH], F32, tag="kv_sb")
    for i in range(SCH):
        pkv = ps.tile([128, 512], F32, tag="ps")
        for c in range(KC):
            nc.tensor.matmul(pkv[:], hT[:, c, i * 128:(i + 1) * 128],
                             wkv_sb[:, c, :], start=(c == 0), stop=(c == KC - 1))
        ev()(kv_sb[:, i, :], pkv[:])
        pq = ps.tile([128, 256], F32, tag="ps")
        for c in range(KC):
            nc.tensor.matmul(pq[:], hT[:, c, i * 128:(i + 1) * 128],
                             wq_sb[:, c, :], start=(c == 0), stop=(c == KC - 1))
        ev()(q_nat[:, i, :], pq[:])
    return q_nat, kv_sb


def _dpfp(nc, sb_head, x2_src, tag_x2, tag_out):
    """relu halves into x2, then rolled multiplies -> [128, SCH, FD] f32r."""
    x2 = sb_head.tile([128, SCH, 128], FMAP, tag=tag_x2, name=tag_x2)
    nc.scalar.activation(x2[:, :, 0:DH], x2_src, AF.Relu)
    nc.scalar.activation(x2[:, :, DH:128], x2_src, AF.Relu, scale=-1.0)
    p = sb_head.tile([128, SCH, FD], FMAP, tag=tag_out, name=tag_out)
    for r in (1, 2, 3):
        base = (r - 1) * 128
        eng = nc.gpsimd if r == 2 else nc.vector
        eng.tensor_tensor(out=p[:, :, base + r:base + 128],
                          in0=x2[:, :, r:128], in1=x2[:, :, 0:128 - r],
                          op=ALU.mult)
        nc.vector.tensor_tensor(out=p[:, :, base:base + r],
                                in0=x2[:, :, 0:r], in1=x2[:, :, 128 - r:128],
                                op=ALU.mult)
    return p


def _emit_attn_head_prep(nc, tc, ev, const, q_nat, kv_sb, kvz_all, sb_head, ps):
    """Phase A: per head, dpfp(k) -> pk -> kvzT -> kvz (F-major)."""
    (_wq_sb, _wkv_sb, _wo_sb, _gb_sb, identr, onesf, _epst, epse, onesrow, identb, identq) = const
    for n in range(NHC):
        pk = _dpfp(nc, sb_head, kv_sb[:, :, n * DH:(n + 1) * DH], "x2", "pk")
        vext = sb_head.tile([128, SCH, DP], FMAP, tag="vext")
        nc.scalar.copy(vext[:, :, 0:DH],
                       kv_sb[:, :, NHC * DH + n * DH:NHC * DH + (n + 1) * DH])
        nc.scalar.copy(vext[:, :, DH:DP], onesf[:])
        pkvz = ps.tile([DP, FD], F32, tag="ps")
        for i in range(SCH):
            nc.tensor.matmul(pkvz[:], vext[:, i, :], pk[:, i, :],
                             start=(i == 0), stop=(i == SCH - 1))
        kvzT = sb_head.tile([DP, FD], FMAP, tag="kvzT")
        nc.scalar.copy(kvzT[:], pkvz[:])
        for r in range(3):
            ptr = ps.tile([128, DP], FMAP, tag="ps")
            nc.tensor.transpose(ptr[:], kvzT[:, r * 128:(r + 1) * 128],
                                identb[0:DP, 0:DP])
            nc.vector.tensor_copy(kvz_all[:, n, r, :], ptr[:])


def _emit_group(nc, tc, ev, io, const, g, q_nat, kvz_all, avT_g, sb_head, sb_ln, ps):
    """Phase B for s-group g: per head dpfp(q)/transpose/num/divide, then
    o-projection, ReduceScatter, residual+LayerNorm for this group."""
    (_h_in, hres_in, out_p, rs_in, rs_out, _r_dram) = io
    (_wq_sb, _wkv_sb, wo_sb, gb_sb, identr, onesf, epst, epse, onesrow, identb, identq) = const

    for n in range(NHC):
        x2q = sb_head.tile([128, 4, 128], QMAP, tag="x2q", bufs=2, name="x2q")
        src = q_nat[:, g * 4:(g + 1) * 4, n * DH:(n + 1) * DH]
        nc.scalar.activation(x2q[:, :, 0:DH], src, AF.Relu)
        nc.scalar.activation(x2q[:, :, DH:128], src, AF.Relu, scale=-1.0)
        pq = sb_head.tile([128, 4, FD], QMAP, tag="pq_g", bufs=2, name="pq")
        for r in (1, 2, 3):
            base = (r - 1) * 128
            eng = nc.gpsimd if r == 2 else nc.vector
            eng.tensor_tensor(out=pq[:, :, base + r:base + 128],
                              in0=x2q[:, :, r:128], in1=x2q[:, :, 0:128 - r],
                              op=ALU.mult)
            nc.vector.tensor_tensor(out=pq[:, :, base:base + r],
                                    in0=x2q[:, :, 0:r], in1=x2q[:, :, 128 - r:128],
                                    op=ALU.mult)
        pqT = sb_head.tile([128, 3, 512], QMAP, tag="pqT", bufs=2)
        for r in range(3):
            ptt = ps.tile([128, 512], QMAP, tag="ps")
            for j in range(4):
                nc.tensor.transpose(ptt[:, j * 128:(j + 1) * 128],
                                    pq[:, j, r * 128:(r + 1) * 128], identq[:])
            (nc.vector.tensor_copy if r < 2 else nc.scalar.copy)(pqT[:, r, :], ptt[:])
        pnd = ps.tile([DH + 1, 512], F32, tag="ps")
        for r in range(3):
            nc.tensor.matmul(pnd[:], kvz_all[:, n, r, 0:DH + 1], pqT[:, r, :],
                             start=(r == 0), stop=(r == 2))
        den = sb_head.tile([1, 512], F32R, tag="den", bufs=4)
        nc.scalar.activation(den[:], pnd[DH:DH + 1, :], AF.Identity,
                             bias=epse[0:1, :])
        pden = ps.tile([64, 512], F32, tag="ps")
        nc.tensor.matmul(pden[:], onesrow[:], den[:], start=True, stop=True)
        r_bc = sb_head.tile([64, 512], F32, tag="r_bc", bufs=4)
        nc.vector.reciprocal(r_bc[:], pden[:])
        nc.vector.tensor_tensor(
            out=avT_g[(n % 2) * 64:(n % 2) * 64 + 64, n // 2, :],
            in0=pnd[0:DH, :], in1=r_bc[:], op=ALU.mult)

    for il in range(4):
        for dg in range(2):
            po = ps.tile([128, 512], F32, tag="ps")
            for m in range(2):
                nc.tensor.matmul(po[:], avT_g[:, m, il * 128:(il + 1) * 128],
                                 wo_sb[:, m, dg * 512:(dg + 1) * 512],
                                 start=(m == 0), stop=(m == 1))
            o_t = sb_ln.tile([128, 512], F32, tag="o_t", bufs=2)
            ev()(o_t[:], po[:])
            nc.sync.dma_start(
                out=rs_in[g][il * 128:(il + 1) * 128, dg * 512:(dg + 1) * 512],
                in_=o_t[:])
    if os.environ.get("NO_COLL"):
        nc.sync.dma_start(out=rs_out[g][:], in_=rs_in[g][0:128, :])
    else:
        nc.gpsimd.collective_compute(
            "ReduceScatter", ALU.add,
            replica_groups=[[0, 1, 2, 3], [4, 5, 6, 7]],
            ins=[rs_in[g][:].opt()],
            outs=[rs_out[g][:].opt()],
        )
    x = sb_ln.tile([128, D], F32, tag="x")
    nc.sync.dma_start(out=x[:], in_=rs_out[g][:])
    hres = sb_ln.tile([128, D], F32, tag="hres")
    nc.sync.dma_start(out=hres[:], in_=hres_in[g])
    nc.gpsimd.tensor_tensor(out=x[:], in0=x[:], in1=hres[:], op=ALU.add)
    stats = sb_ln.tile([128, 2, 6], F32, tag="stats", bufs=2)
    xg = x[:].rearrange("p (s f) -> p s f", s=2)
    for si in range(2):
        nc.vector.bn_stats(out=stats[:, si, :], in_=xg[:, si, :])
    mv = sb_ln.tile([128, 2], F32, tag="mv", bufs=2)
    nc.vector.bn_aggr(out=mv[:], in_=stats[:])
    rstd = sb_ln.tile([128, 1], F32, tag="rstd", bufs=2)
    nc.scalar.activation(rstd[:], mv[:, 1:2], AF.Sqrt, bias=epst[:])
    nc.vector.reciprocal(rstd[:], rstd[:])
    t1 = sb_ln.tile([128, D], F32, tag="t1")
    nc.vector.scalar_tensor_tensor(out=t1[:], in0=x[:], scalar=mv[:, 0:1],
                                   in1=gb_sb[:, 0, :], op0=ALU.subtract,
                                   op1=ALU.mult)
    o_f = sb_ln.tile([128, D], F32, tag="o_f")
    nc.gpsimd.tensor_scalar(out=o_f[:], in0=t1[:], scalar1=rstd[:],
                            scalar2=None, op0=ALU.mult)
    nc.gpsimd.tensor_tensor(out=o_f[:], in0=o_f[:], in1=gb_sb[:, 1, :],
                            op=ALU.add)
    nc.sync.dma_start(out=out_p[g], in_=o_f[:])


def build(reps=1):
    nc = bacc.Bacc("TRN2", target_bir_lowering=False, debug=False)

    h_in = nc.dram_tensor("h", [S, D], F32, kind="ExternalInput")
    hres_in = nc.dram_tensor("hres", [SG, 128, D], F32, kind="ExternalInput")
    wq_in = nc.dram_tensor("wq", [D, NHC * DH], F32, kind="ExternalInput")
    wkv_in = nc.dram_tensor("wkv", [D, 2 * NHC * DH], F32, kind="ExternalInput")
    wo_in = nc.dram_tensor("wo", [NHC * DH, D], F32, kind="ExternalInput")
    gamma_in = nc.dram_tensor("gamma", [D], F32, kind="ExternalInput")
    beta_in = nc.dram_tensor("beta", [D], F32, kind="ExternalInput")
    out_p = nc.dram_tensor("out", [SG, 128, D], F32, kind="ExternalOutput")

    rs_in = [nc.dram_tensor(f"rs_bounce_in{g}", [512, D], F32) for g in range(SG)]
    rs_out = [nc.dram_tensor(f"rs_bounce_out{g}", [128, D], F32) for g in range(SG)]
    io = (h_in, hres_in, out_p, rs_in, rs_out, None)

    ev_state = [0]

    with tile.TileContext(nc) as tc:
        def ev():
            ev_state[0] += 1
            if ev_state[0] % 2:
                return nc.vector.tensor_copy
            return nc.scalar.copy

        with contextlib.ExitStack() as ctx:
            sb_c = ctx.enter_context(tc.tile_pool(name="const", bufs=1))
            ps = ctx.enter_context(tc.tile_pool(name="psum", bufs=7, space="PSUM"))
            sb_qkv = ctx.enter_context(tc.tile_pool(name="qkv", bufs=1))
            sb_attn = ctx.enter_context(tc.tile_pool(name="attn", bufs=1))

            wq_sb = sb_c.tile([128, KC, NHC * DH], F32R)
            nc.gpsimd.dma_start(out=wq_sb[:],
                                in_=wq_in[:].rearrange("(c p) m -> p c m", p=128))
            wkv_sb = sb_c.tile([128, KC, 2 * NHC * DH], F32R)
            nc.gpsimd.dma_start(out=wkv_sb[:],
                                in_=wkv_in[:].rearrange("(c p) m -> p c m", p=128))
            wo_sb = sb_c.tile([128, 2, D], F32R)
            nc.gpsimd.dma_start(out=wo_sb[:],
                                in_=wo_in[:].rearrange("(c p) m -> p c m", p=128))
            gb_sb = sb_c.tile([128, 2, D], F32)
            nc.sync.dma_start(out=gb_sb[:, 0, :],
                              in_=gamma_in[:].partition_broadcast(128))
            nc.sync.dma_start(out=gb_sb[:, 1, :],
                              in_=beta_in[:].partition_broadcast(128))
            ident = sb_c.tile([128, 128], F32)
            nc.gpsimd.memset(ident[:], 0.0)
            nc.gpsimd.affine_select(out=ident[:], in_=ident[:],
                                    compare_op=ALU.not_equal, fill=1.0, base=0,
                                    pattern=[[-1, 128]], channel_multiplier=1)
            identr = sb_c.tile([128, 128], F32R)
            nc.scalar.copy(identr[:], ident[:])
            identb = sb_c.tile([128, 128], FMAP)
            nc.scalar.copy(identb[:], ident[:])
            identq = sb_c.tile([128, 128], QMAP)
            nc.scalar.copy(identq[:], ident[:])
            onesf = sb_c.tile([128, SCH, DP - DH], F32)
            nc.vector.memset(onesf[:], 1.0)
            epst = sb_c.tile([128, 1], F32)
            nc.vector.memset(epst[:], 1e-5)
            epse = sb_c.tile([128, 1], F32)
            nc.vector.memset(epse[:], EPS_EFF)
            onesrow_f = sb_c.tile([1, 64], F32)
            nc.vector.memset(onesrow_f[:], 1.0)
            onesrow = sb_c.tile([1, 64], F32R)
            nc.scalar.copy(onesrow[:], onesrow_f[:])
            const = (wq_sb, wkv_sb, wo_sb, gb_sb, identr, onesf, epst, epse, onesrow, identb, identq)

            phases = os.environ.get("PHASES", "full")
            for _rep in range(reps):
                with tc.tile_pool(name="hT", bufs=1) as sb_hT, \
                     tc.tile_pool(name="hld", bufs=1) as sb_hld:
                    q_nat, kv_sb = _emit_proj(nc, tc, ev, io, const,
                                              sb_hT, sb_hld, sb_qkv, ps)
                if phases == "proj":
                    for g in range(SG):
                        nc.sync.dma_start(out=out_p[g][:, 0:512],
                                          in_=kv_sb[:, g, :])
                    continue
                with tc.tile_pool(name="head", bufs=1) as sb_head, \
                     tc.tile_pool(name="ln", bufs=1) as sb_ln:
                    kvz_all = sb_head.tile([128, NHC, 3, DP], QMAP, tag="kvz_all")
                    _emit_attn_head_prep(nc, tc, ev, const, q_nat, kv_sb,
                                         kvz_all, sb_head, ps)
                    for g in range(SG):
                        avT_g = sb_attn.tile([128, NHC // 2, 512], F32R,
                                             tag="avT", bufs=2)
                        _emit_group(nc, tc, ev, io, const, g, q_nat, kvz_all,
                                    avT_g, sb_head, sb_ln, ps)
    nc.compile()
    return nc


_NC_CACHE = {}


def _get_nc(reps=1):
    if reps not in _NC_CACHE:
        _NC_CACHE[reps] = build(reps)
    return _NC_CACHE[reps]


def make_in_maps(h, Wq, Wkv, Wo, ln_gamma, ln_beta):
    h = np.asarray(h, dtype=np.float32)
    Wq = np.asarray(Wq, dtype=np.float32)
    Wkv = np.asarray(Wkv, dtype=np.float32)
    Wo = np.asarray(Wo, dtype=np.float32)
    g = np.ascontiguousarray(np.asarray(ln_gamma, dtype=np.float32))
    be = np.ascontiguousarray(np.asarray(ln_beta, dtype=np.float32))
    in_maps = []
    for c in range(N_CORES):
        b, hg, r = c // GRP, c % GRP, c % GRP
        h_b = np.ascontiguousarray(h[:, b, :])
        hres = np.stack([h_b[gi * 512 + r * 128: gi * 512 + (r + 1) * 128]
                         for gi in range(SG)])
        cs = hg * NHC * DH
        in_maps.append({
            "h": h_b,
            "hres": np.ascontiguousarray(hres),
            "wq": np.ascontiguousarray(Wq[:, cs:cs + 256]),
            "wkv": np.ascontiguousarray(
                np.concatenate([Wkv[:, cs:cs + 256],
                                Wkv[:, NH * DH + cs:NH * DH + cs + 256]],
                               axis=1)),
            "wo": np.ascontiguousarray(Wo[cs:cs + 256, :]),
            "gamma": g,
            "beta": be,
        })
    return in_maps


def assemble(results):
    out = np.empty((S, B, D), dtype=np.float32)
    for c in range(N_CORES):
        b, r = c // GRP, c % GRP
        o = results[c]["out"]
        for gi in range(SG):
            out[gi * 512 + r * 128: gi * 512 + (r + 1) * 128, b, :] = o[gi]
    return out


def run(in_maps, reps=1):
    nc = _get_nc(reps)
    return run_bass_kernel_spmd(nc, in_maps, list(range(N_CORES)))


def kernel(h, Wq, Wkv, Wo, ln_gamma, ln_beta):
    in_maps = make_in_maps(h, Wq, Wkv, Wo, ln_gamma, ln_beta)
    # The first execution right after a fresh compile occasionally hits a
    # transient "mesh desynced" collective error while the NEFF is still
    # loading on some cores; a retry on the (now cached) NEFF succeeds.
    last = None
    for _ in range(3):
        try:
            res = run(in_maps, reps=1)
            return assemble(res.results)
        except Exception as e:  # noqa: BLE001
            last = e
    raise last



# revision 2
# speedup vs baseline: 31.3985x; 31.3985x over previous
"""DPFP multi-head linear attention for 8 Trainium2 NeuronCores, v2.

Sharding: data-parallel over batch (B=2 -> two groups of 4 cores), and
sequence-parallel WITHIN each group (each core owns 512 of 2048 rows for
ALL 16 heads).  The only cross-core exchange is the kvz aggregate
(sum_j pk_j (x) [v_j|1], shape [384, 16, 68] bf16 per core), reduced with
a ReduceScatter + AllGather pair over each 4-core group.  The o-projection
needs no collective: every core holds all head dims for its own rows.

Math (per head n):
  q = h @ Wq, k,v = split(h @ Wkv)
  pq = dpfp(q), pk = dpfp(k)            # dpfp: x2=[relu(x), relu(-x)];
                                        # f=(r-1)*128+j -> x2[j]*x2[(j-r)%128]
  kvz[f, d] = sum_j pk[j, f] * [v|1][j, d]     # global sum over j -> collective
  num/den   = kvz^T-contract-f pq^T            # per local row
  out = LN(h + (num/den) @ Wo)

Feature maps, weights and collective payloads are bf16; matmuls run at the
full PE rate either way, and the final LayerNorm + the num/den ratio keep
the output error ~1e-3, well under the 2e-2 gate.

The pq^T needed by the num matmul is built without per-head PE transposes of
the full feature map: x2 is transposed once per head, and the dpfp "roll"
is a matmul with a 128x128 roll-permutation matrix M_r, so the rolled copy
lands partition-aligned with x2^T and a plain tensor_tensor multiply
produces pq^T directly.
"""
import contextlib
import os
import sys

sys.path.insert(0, "/opt/trn_rl_repo")

import numpy as np

import concourse.bacc as bacc
import concourse.mybir as mybir
import concourse.tile as tile
from concourse.bass_utils import run_bass_kernel_spmd

AF = mybir.ActivationFunctionType
ALU = mybir.AluOpType
F32 = mybir.dt.float32
BF16 = mybir.dt.bfloat16
NPBF16 = mybir.dt.np(mybir.dt.bfloat16)

S = 2048
B = 2
D = 1024
NH = 16
DH = 64
FD = 384          # dpfp feature dim per head
SL = 512          # local sequence rows per core
SC = SL // 128    # 4 local s-chunks
KC = D // 128     # 8 contraction chunks
DP = 68           # padded [v|1] width
EPS_EFF = 8e-5    # EPS / SCALE = 1e-5 * sqrt(64)
N_CORES = 8
GRP = 4
REPLICA_GROUPS = [[0, 1, 2, 3], [4, 5, 6, 7]]


def _emit_rep(nc, tc, const, io, ps, sb_rep, rep=0):
    (h_in, wq_sb, wkv_sb, wo_sb, gb_sb, identf, identb, mrolls, onescol,
     epst, epse, mroll_sb) = const
    (kvz_part, kvz_red, kvz_full, out_p) = io

    # ---- persistent per-rep tiles ----
    h_nat = sb_rep.tile([128, SC, D], F32, tag="h_nat")
    kvz_sb = sb_rep.tile([128, 3, NH, DP], BF16, tag="kvz_sb")
    pqT = sb_rep.tile([128, NH, 3, SL], BF16, tag="pqT")
    avT = sb_rep.tile([128, NH // 2, SL], BF16, tag="avT")

    # input DMAs: h + k-half of wkv first on the shared DMA device
    h_re = h_in[:].rearrange("(c p) m -> p c m", p=128)
    nc.sync.dma_start(out=h_nat[:, 0, :], in_=h_re[:, 0, :])
    if rep == 0:
        nc.sync.dma_start(out=wkv_sb[0][:, :, 0:D], in_=wkv_sb[1])
    for sc in range(1, SC):
        nc.sync.dma_start(out=h_nat[:, sc, :], in_=h_re[:, sc, :])
    wkv = wkv_sb[0]
    wq = wq_sb[0]
    wo = wo_sb[0]

    with contextlib.ExitStack() as ctx:
        ctx.enter_context(nc.allow_low_precision(
            reason="bf16 feature maps; num/den ratio cancels rounding"))
        sb_hT = ctx.enter_context(tc.tile_pool(name="hT", bufs=1))
        sb_kv = ctx.enter_context(tc.tile_pool(name="kv", bufs=1))

        # ---- h transpose (interleaved with k-proj below) ----
        hT = sb_hT.tile([128, KC, SL], BF16, tag="hT")

        def emit_transpose(sc):
            for cg in range(2):
                pt = ps.tile([128, 4, 128], F32, tag="ps", bufs=8)
                for j in range(4):
                    c = cg * 4 + j
                    nc.tensor.transpose(
                        pt[:, j, :], h_nat[:, sc, c * 128:(c + 1) * 128],
                        identf[:])
                nc.scalar.copy(
                    hT[:, cg * 4:cg * 4 + 4, sc * 128:(sc + 1) * 128], pt[:])

        x2k = sb_kv.tile([128, SC, NH, 128], BF16, tag="x2k")
        vext = sb_kv.tile([128, SC, NH, DP], BF16, tag="vext")
        nc.vector.memset(vext[:, :, :, DH + 1:DP], 0.0)
        nc.vector.memset(vext[:, :, :, DH:DH + 1], 1.0)

        def emit_kv(sc, half, is_v):
            pp = ps.tile([128, 512], F32, tag="ps", bufs=8)
            off = D if is_v else 0
            for c in range(KC):
                nc.tensor.matmul(
                    pp[:],
                    hT[:, c, sc * 128:(sc + 1) * 128],
                    wkv[:, c, off + half * 512:off + (half + 1) * 512],
                    start=(c == 0), stop=(c == KC - 1))
            re = pp[:].rearrange("p (n d) -> p n d", n=8)
            hs = slice(half * 8, half * 8 + 8)
            if is_v:
                nc.scalar.copy(vext[:, sc, hs, 0:DH], re)
            else:
                nc.scalar.activation(x2k[:, sc, hs, 0:DH], re, AF.Relu)
                nc.scalar.activation(x2k[:, sc, hs, DH:128], re, AF.Relu,
                                     scale=-1.0)

        def emit_head(n):
            pk = sb_kv.tile([128, SC, FD], BF16, tag="pk", bufs=2)
            for r in (1, 2, 3):
                base = (r - 1) * 128
                eng = nc.vector if r == 1 else nc.gpsimd
                eng.tensor_tensor(out=pk[:, :, base + r:base + 128],
                                  in0=x2k[:, :, n, r:128],
                                  in1=x2k[:, :, n, 0:128 - r],
                                  op=ALU.mult)
                nc.vector.tensor_tensor(out=pk[:, :, base:base + r],
                                        in0=x2k[:, :, n, 0:r],
                                        in1=x2k[:, :, n, 128 - r:128],
                                        op=ALU.mult)
            kzp = ps.tile([128, 3, DP], F32, tag="ps", bufs=8)
            for r in range(3):
                for sc in range(SC):
                    nc.tensor.matmul(kzp[:, r, :],
                                     pk[:, sc, r * 128:(r + 1) * 128],
                                     vext[:, sc, n, :],
                                     start=(sc == 0), stop=(sc == SC - 1))
            nc.scalar.copy(kvz_sb[:, :, n, :], kzp[:])

        emit_transpose(0)
        for sc in range(SC):
            emit_kv(sc, 0, False)
            if sc + 1 < SC:
                emit_transpose(sc + 1)
        if rep == 0:
            nc.scalar.dma_start(out=wkv_sb[0][:, :, D:2 * D], in_=wkv_sb[2])
            nc.scalar.dma_start(out=wq_sb[0][:], in_=wq_sb[1])
            nc.scalar.dma_start(out=mroll_sb[0][:], in_=mroll_sb[1])
        for sc in range(SC):
            emit_kv(sc, 0, True)
        for n in range(NH // 2):
            emit_head(n)
        for sc in range(SC):
            emit_kv(sc, 1, False)
        for sc in range(SC):
            emit_kv(sc, 1, True)
        for n in range(NH // 2, NH):
            emit_head(n)

        if rep == 0:
            nc.gpsimd.dma_start(out=wo_sb[0][:], in_=wo_sb[1])
            nc.gpsimd.dma_start(out=gb_sb[0][:, 0, :], in_=gb_sb[1])
            nc.gpsimd.dma_start(out=gb_sb[0][:, 1, :], in_=gb_sb[2])

        # ---- kvz exchange: RS + AG over the 4-core group ----
        nc.sync.dma_start(
            out=kvz_part[:].rearrange("(r p) h c -> p r h c", p=128),
            in_=kvz_sb[:])
        if os.environ.get("NO_COLL"):
            nc.sync.dma_start(out=kvz_red[:], in_=kvz_part[0:96])
            nc.sync.dma_start(
                out=kvz_full[:].rearrange("(g x) h c -> g x h c", g=4),
                in_=kvz_part[:].rearrange("(g x) h c -> g x h c", g=4))
        else:
            nc.gpsimd.collective_compute(
                "ReduceScatter", ALU.add, replica_groups=REPLICA_GROUPS,
                ins=[kvz_part[:].opt()], outs=[kvz_red[:].opt()])
            nc.gpsimd.collective_compute(
                "AllGather", ALU.bypass, replica_groups=REPLICA_GROUPS,
                ins=[kvz_red[:].opt()], outs=[kvz_full[:].opt()])
        nc.sync.dma_start(
            out=kvz_sb[:],
            in_=kvz_full[:].rearrange("(r p) h c -> p r h c", p=128))

        # ---- q side (overlaps the collective): proj, relu, pqT ----
        x2q = sb_kv.tile([128, SC, NH, 128], BF16, tag="x2k")
        for sc in range(SC):
            for half in range(2):
                qp = ps.tile([128, 512], F32, tag="ps", bufs=8)
                for c in range(KC):
                    nc.tensor.matmul(
                        qp[:],
                        hT[:, c, sc * 128:(sc + 1) * 128],
                        wq[:, c, half * 512:(half + 1) * 512],
                        start=(c == 0), stop=(c == KC - 1))
                qre = qp[:].rearrange("p (n d) -> p n d", n=8)
                hs = slice(half * 8, half * 8 + 8)
                nc.scalar.activation(x2q[:, sc, hs, 0:DH], qre, AF.Relu)
                nc.scalar.activation(x2q[:, sc, hs, DH:128], qre, AF.Relu,
                                     scale=-1.0)
        for n in range(NH):
            x2tp = ps.tile([128, SL], BF16, tag="ps", bufs=8)
            for sc in range(SC):
                nc.tensor.transpose(x2tp[:, sc * 128:(sc + 1) * 128],
                                    x2q[:, sc, n, :], identb[:])
            x2T = sb_kv.tile([128, SL], BF16, tag="x2T", bufs=2)
            nc.vector.tensor_copy(x2T[:], x2tp[:])
            for r in (1, 2, 3):
                rp = ps.tile([128, SL], F32, tag="ps", bufs=8)
                nc.tensor.matmul(rp[:], mrolls[r - 1], x2T[:],
                                 start=True, stop=True)
                nc.vector.tensor_tensor(out=pqT[:, n, r - 1, :], in0=rp[:],
                                        in1=x2T[:], op=ALU.mult)

        # ---- num/den per head -> avT ----
        LAG = 3
        pnds = {}
        for nn in range(NH + LAG):
            if nn < NH:
                n = nn
                pnd = ps.tile([DH + 1, SL], F32, tag="ps", bufs=8)
                for r in range(3):
                    nc.tensor.matmul(pnd[:], kvz_sb[:, r, n, 0:DH + 1],
                                     pqT[:, n, r, :], start=(r == 0),
                                     stop=(r == 2))
                den = sb_kv.tile([1, SL], BF16, tag="den", bufs=2)
                nc.scalar.activation(den[:], pnd[DH:DH + 1, :], AF.Identity,
                                     bias=epse[0:1, :])
                rden = sb_kv.tile([1, SL], BF16, tag="rden", bufs=2)
                nc.vector.reciprocal(rden[:], den[:])
                pnds[n] = (pnd, rden)
            if nn >= LAG:
                n = nn - LAG
                pnd, rden = pnds.pop(n)
                dbp = ps.tile([DH, SL], F32, tag="ps", bufs=8)
                nc.tensor.matmul(dbp[:], onescol[:], rden[:], start=True,
                                 stop=True)
                dbpc = sb_kv.tile([DH, SL], BF16, tag="dbpc", bufs=2)
                nc.scalar.copy(dbpc[:], dbp[:])
                nc.vector.tensor_tensor(
                    out=avT[(n % 2) * DH:(n % 2) * DH + DH, n // 2, :],
                    in0=pnd[0:DH, :], in1=dbpc[:], op=ALU.mult)

        # ---- o-projection + residual + LayerNorm per s-chunk ----
        for sc in range(SC):
            x = h_nat[:, sc, :]
            for half in range(2):
                op = ps.tile([128, 512], F32, tag="ps", bufs=8)
                for hc in range(8):
                    nc.tensor.matmul(
                        op[:],
                        avT[:, hc, sc * 128:(sc + 1) * 128],
                        wo[:, hc, half * 512:(half + 1) * 512],
                        start=(hc == 0), stop=(hc == 7))
                xh = h_nat[:, sc, half * 512:(half + 1) * 512]
                nc.vector.tensor_tensor(out=xh, in0=op[:], in1=xh,
                                        op=ALU.add)
            stats = sb_kv.tile([128, 2, 6], F32, tag="stats", bufs=2)
            xg = x.rearrange("p (s f) -> p s f", s=2)
            for si in range(2):
                nc.vector.bn_stats(out=stats[:, si, :], in_=xg[:, si, :])
            mv = sb_kv.tile([128, 2], F32, tag="mv", bufs=2)
            nc.vector.bn_aggr(out=mv[:], in_=stats[:])
            rstd = sb_kv.tile([128, 1], F32, tag="rstd", bufs=2)
            nc.scalar.activation(rstd[:], mv[:, 1:2], AF.Sqrt, bias=epst[:])
            nc.vector.reciprocal(rstd[:], rstd[:])
            for half in range(2):
                hs = slice(half * 512, (half + 1) * 512)
                t1 = sb_kv.tile([128, 512], F32, tag="t1", bufs=2)
                nc.vector.scalar_tensor_tensor(out=t1[:], in0=x[:, hs],
                                               scalar=mv[:, 0:1],
                                               in1=gb_sb[0][:, 0, hs],
                                               op0=ALU.subtract,
                                               op1=ALU.mult)
                o_f = sb_kv.tile([128, 512], F32, tag="o_f", bufs=2)
                nc.gpsimd.tensor_scalar(out=o_f[:], in0=t1[:],
                                        scalar1=rstd[:], scalar2=None,
                                        op0=ALU.mult)
                nc.gpsimd.tensor_tensor(out=o_f[:], in0=o_f[:],
                                        in1=gb_sb[0][:, 1, hs], op=ALU.add)
                nc.sync.dma_start(out=out_p[sc * 128:(sc + 1) * 128, hs],
                                  in_=o_f[:])


def build(reps=1):
    nc = bacc.Bacc("TRN2", target_bir_lowering=False, debug=False)

    h_in = nc.dram_tensor("h", [SL, D], F32, kind="ExternalInput")
    wq_in = nc.dram_tensor("wq", [D, D], BF16, kind="ExternalInput")
    wkv_in = nc.dram_tensor("wkv", [D, 2 * D], BF16, kind="ExternalInput")
    wo_in = nc.dram_tensor("wo", [D, D], BF16, kind="ExternalInput")
    gamma_in = nc.dram_tensor("gamma", [D], F32, kind="ExternalInput")
    beta_in = nc.dram_tensor("beta", [D], F32, kind="ExternalInput")
    identf_in = nc.dram_tensor("identf", [128, 128], F32,
                               kind="ExternalInput")
    mroll_in = nc.dram_tensor("mroll", [3, 128, 128], BF16,
                              kind="ExternalInput")
    out_p = nc.dram_tensor("out", [SL, D], F32, kind="ExternalOutput")

    kvz_part = nc.dram_tensor("kvz_part", [3 * 128, NH, DP], BF16)
    kvz_red = nc.dram_tensor("kvz_red", [3 * 128 // GRP, NH, DP], BF16)
    kvz_full = nc.dram_tensor("kvz_full", [3 * 128, NH, DP], BF16)
    io = (kvz_part, kvz_red, kvz_full, out_p)

    with tile.TileContext(nc) as tc:
        with contextlib.ExitStack() as ctx:
            sb_c = ctx.enter_context(tc.tile_pool(name="const", bufs=1))
            ps = ctx.enter_context(tc.tile_pool(name="psum", bufs=8,
                                                space="PSUM"))
            sb_rep = ctx.enter_context(tc.tile_pool(name="rep", bufs=1))

            # weight tiles are persistent; their DMAs are issued inside the
            # rep loop (first rep only) so rep timing includes the loads only
            # once, matching the baseline convention.
            wq_t = sb_c.tile([128, KC, D], BF16)
            wkv_t = sb_c.tile([128, KC, 2 * D], BF16)
            wo_t = sb_c.tile([128, KC, D], BF16)
            gb_t = sb_c.tile([128, 2, D], F32)
            gb_sb = (gb_t, gamma_in[:].partition_broadcast(128),
                     beta_in[:].partition_broadcast(128))
            identf = sb_c.tile([128, 128], F32)
            nc.sync.dma_start(out=identf[:], in_=identf_in[:])
            identb = sb_c.tile([128, 128], BF16)
            nc.scalar.copy(identb[:], identf[:])
            mroll_t = sb_c.tile([128, 3, 128], BF16)
            mroll_sb = (mroll_t, mroll_in[:].rearrange("r p f -> p r f"))
            mrolls = [mroll_t[:, r, :] for r in range(3)]
            onescol_f = sb_c.tile([1, DH], F32)
            nc.vector.memset(onescol_f[:], 1.0)
            onescol = sb_c.tile([1, DH], BF16)
            nc.scalar.copy(onescol[:], onescol_f[:])
            epst = sb_c.tile([128, 1], F32)
            nc.vector.memset(epst[:], 1e-5)
            epse = sb_c.tile([1, 1], F32)
            nc.vector.memset(epse[:], EPS_EFF)

            wq_sb = (wq_t, wq_in[:].rearrange("(c p) m -> p c m", p=128))
            wkv_k = wkv_in[:, 0:D].rearrange("(c p) m -> p c m", p=128)
            wkv_v = wkv_in[:, D:2 * D].rearrange("(c p) m -> p c m", p=128)
            wkv_sb = (wkv_t, wkv_k, wkv_v)
            wo_sb = (wo_t, wo_in[:].rearrange("(c p) m -> p c m", p=128))
            const = (h_in, wq_sb, wkv_sb, wo_sb, gb_sb, identf, identb,
                     mrolls, onescol, epst, epse, mroll_sb)

            for _rep in range(reps):
                _emit_rep(nc, tc, const, io, ps, sb_rep, rep=_rep)
    nc.compile()
    return nc


_NC_CACHE = {}


def _get_nc(reps=1):
    if reps not in _NC_CACHE:
        _NC_CACHE[reps] = build(reps)
    return _NC_CACHE[reps]


_IDENTF = np.eye(128, dtype=np.float32)
_MROLL = np.stack([
    np.eye(128, k=r, dtype=np.float32) + np.eye(128, k=r - 128,
                                                dtype=np.float32)
    for r in (1, 2, 3)
]).astype(NPBF16)  # M_r[p, f] = 1 iff p == (f - r) mod 128


def make_in_maps(h, Wq, Wkv, Wo, ln_gamma, ln_beta):
    h = np.asarray(h, dtype=np.float32)
    wq = np.asarray(Wq, dtype=np.float32).astype(NPBF16)
    wkv = np.asarray(Wkv, dtype=np.float32).astype(NPBF16)
    wo = np.asarray(Wo, dtype=np.float32).astype(NPBF16)
    g = np.ascontiguousarray(np.asarray(ln_gamma, dtype=np.float32))
    be = np.ascontiguousarray(np.asarray(ln_beta, dtype=np.float32))
    # Wkv columns: reference splits [k(16 heads x 64) | v(...)]; our layout
    # wants [k | v] too, which Wkv already is.
    in_maps = []
    for c in range(N_CORES):
        b, r = c // GRP, c % GRP
        in_maps.append({
            "h": np.ascontiguousarray(h[r * SL:(r + 1) * SL, b, :]),
            "wq": np.ascontiguousarray(wq),
            "wkv": np.ascontiguousarray(wkv),
            "wo": np.ascontiguousarray(wo),
            "gamma": g,
            "beta": be,
            "identf": _IDENTF,
            "mroll": _MROLL,
        })
    return in_maps


def assemble(results):
    out = np.empty((S, B, D), dtype=np.float32)
    for c in range(N_CORES):
        b, r = c // GRP, c % GRP
        out[r * SL:(r + 1) * SL, b, :] = results[c]["out"]
    return out


def run(in_maps, reps=1):
    nc = _get_nc(reps)
    return run_bass_kernel_spmd(nc, in_maps, list(range(N_CORES)))


def kernel(h, Wq, Wkv, Wo, ln_gamma, ln_beta):
    in_maps = make_in_maps(h, Wq, Wkv, Wo, ln_gamma, ln_beta)
    last = None
    for _ in range(3):
        try:
            res = run(in_maps, reps=1)
            return assemble(res.results)
        except Exception as e:  # noqa: BLE001
            last = e
    raise last
